# revision 1
# baseline (speedup 1.0000x reference)
"""Distributed causal multi-head attention for Trainium2 (8 NeuronCores).

Problem (nn_Attention): B=2, T=2048, D=2048, H=16 heads, d_head=128.
  q/k/v = x @ {q,k,v}_out; per-head causal softmax attention; out = ctx @ w_out.

Sharding: batch (2) x head-group (4 heads each) -> 8 cores. Each core computes
its batch's attention for its 4 heads plus the partial output projection
(w_out row-sharded); the host sums the 4 partials per batch (all-reduce) and
stacks batches.

All device tensors are fp16 (host converts): PE rate is identical to fp32r
(1 cyc/row) but every DMA halves, the <256-wide fp32r penalty disappears,
and the whole working set (x^T, V, Q^T/K^T, all four heads' context, w_out
slice) fits in SBUF -- no DRAM scratch round-trip. PSUM accumulation stays
fp32 throughout, so precision loss is only input/intermediate quantization
(~1e-3 rel).

Per-core kernel (all matmuls fp16, moving free dim 512):
  phase 1: V  = x @ wv   (single pass, wv resident, all 4 heads at once)
  phase 2: per head: Q^T, K^T projections; then flash-style causal attention
           with scores kept transposed (tk on partitions):
             S^T chunk = K^T_chunk.T @ Q^T_tile        (PE)
             P^T = exp(S^T / sqrt(dh))                 (ACT, PSUM->SBUF, f16)
             diagonal chunks masked via precomputed 0/1 mask    (DVE)
             C^T += V_chunk.T @ P^T ; Z += ones.T @ P^T (PE, PSUM accum)
             C^T_norm = C^T * 1/Z -> SBUF ct_sb        (DVE)
  phase 3: out[tq, :] += sum_h C_h @ wo_h  (PSUM accum over heads),
           interleaved with head-3 attention per tq-group.

Host passes x^T (per batch) so the contraction dim D is on partitions
everywhere; no on-device transposes needed anywhere.
"""

import math

import numpy as np

import concourse.bacc as bacc
import concourse.mybir as mybir
import concourse.tile as tile
from concourse.bass_utils import run_bass_kernel_spmd

# ---- problem constants (hardcoded; self-contained) ----
B = 2
T = 2048
D = 2048
H_PER = 4            # heads per core
DH = 128             # head dim
GCOLS = H_PER * DH   # 512 columns per head-group
P = 128
KC = D // P          # 16 contraction chunks
TT = 512             # t tile (matmul moving free dim)
NTT = T // TT        # 4
NTCH = T // P        # 16 t chunks
DOT = 512            # output-dim tile
NDOT = D // DOT      # 4

F32 = mybir.dt.float32
F16 = mybir.dt.float16
SCALE = 1.0 / math.sqrt(float(DH))

_CACHE = {}


def _build(n_repeat=1):
    nc = bacc.Bacc("TRN2", target_bir_lowering=False, debug=False)
    xT_d = nc.dram_tensor("xT", (D, T), F16, kind="ExternalInput")
    # wq/wk are host-packed per head into the exact SBUF tile layout
    # (p, c, dh): a column-sliced (D, 128) load would have 256B descriptors
    # and pay the <512B DMA read-modify-write penalty
    wq_d = nc.dram_tensor("wq", (H_PER, P, KC, DH), F16, kind="ExternalInput")
    wk_d = nc.dram_tensor("wk", (H_PER, P, KC, DH), F16, kind="ExternalInput")
    wv_d = nc.dram_tensor("wv", (D, GCOLS), F16, kind="ExternalInput")
    wo_d = nc.dram_tensor("wo", (GCOLS, D), F16, kind="ExternalInput")
    out_d = nc.dram_tensor("out", (T, D), F16, kind="ExternalOutput")

    xT_r = xT_d.ap().rearrange("(c p) t -> p c t", p=P)      # (128, 16, 2048)
    wq_r = wq_d.ap()                                         # (4, 128, 16, 128)
    wk_r = wk_d.ap()
    wv_r = wv_d.ap().rearrange("(c p) g -> p c g", p=P)
    wo_r = wo_d.ap().rearrange("(h p) n -> p h n", p=P)      # (128, 4, 2048)
    out_r = out_d.ap()

    with tile.TileContext(nc) as tc:
        with (
            tc.tile_pool(name="const", bufs=1) as const_pool,
            tc.tile_pool(name="big", bufs=1) as big_pool,
            tc.tile_pool(name="wvp", bufs=1) as wv_pool,
            tc.tile_pool(name="vp", bufs=1) as v_pool,
            tc.tile_pool(name="ctp", bufs=1) as ct_pool,
            tc.tile_pool(name="wop", bufs=1) as wo_pool,
            tc.tile_pool(name="qk", bufs=2) as qk_pool,
            tc.tile_pool(name="wqk", bufs=4) as wqk_pool,
            tc.tile_pool(name="work", bufs=6) as work_pool,
        ):
            # ---- constants ----
            # ones for the Z (softmax denominator) matmuls: generated on the
            # idle Pool engine instead of DMA'd, keeping the critical phase-1
            # chunk stream free of an extra HWDGE dispatch
            ones_t = const_pool.tile([P, P], F16, tag="ones", name="ones_t")
            nc.gpsimd.memset(ones_t[:], 1.0)
            # band-local triangular mask: band[p, jj] = 1.0 iff jj >= p
            # (ragged diagonal chunks only ever mask a 128-wide band)
            bandt = const_pool.tile([P, P], F32, tag="mask", name="bandt")
            nc.gpsimd.memset(bandt[:], 1.0)
            nc.gpsimd.affine_select(
                out=bandt[:],
                in_=bandt[:],
                compare_op=mybir.AluOpType.is_ge,
                fill=0.0,
                base=0,
                pattern=[[1, P]],
                channel_multiplier=-1,
            )
            band16 = const_pool.tile([P, P], F16, tag="mask16", name="band16")
            nc.vector.tensor_copy(out=band16[:], in_=bandt[:])
            band_mask = band16[:]

            # warm the ACT exp table during phase 1 (LoadActFuncSet is ~1.3us
            # and otherwise stalls the first real exp)
            actwarm = const_pool.tile([P, 1], F32, tag="actwarm", name="actwarm")
            nc.scalar.activation(
                actwarm[:], bandt[:, 0:1], mybir.ActivationFunctionType.Exp
            )

            for rep in range(n_repeat):
                _emit_body(
                    nc, tc, rep, big_pool, wv_pool, v_pool, ct_pool, wo_pool,
                    qk_pool, wqk_pool, work_pool,
                    ones_t, band_mask,
                    xT_r, wq_r, wk_r, wv_r, wo_r, out_r,
                )

    nc.compile()
    return nc


def _emit_body(nc, tc, rep, big_pool, wv_pool, v_pool, ct_pool, wo_pool,
               qk_pool, wqk_pool, work_pool,
               ones_t, band_mask,
               xT_r, wq_r, wk_r, wv_r, wo_r, out_r):
    R = f"r{rep}_"

    # ---- resident tensors ----
    xT_t = big_pool.tile([P, KC, T], F16, tag="big", name=f"{R}xT_t")
    wv_t = wv_pool.tile([P, KC, GCOLS], F16, tag="wv", name=f"{R}wv_t")
    v_all = v_pool.tile([P, NTCH, GCOLS], F16, tag="v", name=f"{R}v_all")
    ct_sb = ct_pool.tile([P, H_PER, T], F16, tag="ct", name=f"{R}ct_sb")
    wo_t = wo_pool.tile([P, H_PER, D], F16, tag="wo", name=f"{R}wo_t")

    # psQK doubles as the phase-3 accumulator pool (same tag/shape rotates)
    psQK = tc.alloc_tile_pool(name=f"{R}psQK", bufs=3, space="PSUM")

    # per-head weight tiles; all loads on the sync queue at controlled
    # positions so they never cut ahead of the critical phase-1 chunk stream
    wq_ts, wk_ts = [], []
    for h in range(H_PER):
        wq_ts.append(wqk_pool.tile([P, KC, DH], F16, tag="wq", name=f"{R}wq_{h}"))
        wk_ts.append(wqk_pool.tile([P, KC, DH], F16, tag="wk", name=f"{R}wk_{h}"))

    # ---------- phase 1: V = x @ wv (single pass, wv resident) ----------
    psV = tc.alloc_tile_pool(name=f"{R}psV", bufs=1, space="PSUM")
    for k in range(KC):
        # interleave wv + xT chunk loads so phase-1 matmuls can start
        # as soon as the first chunks land (wv batched in pairs after the
        # first chunks: fewer HWDGE dispatch slots in the critical stream)
        if k < 2:
            nc.sync.dma_start(wv_t[:, k], wv_r[:, k])
        elif k % 2 == 0:
            nc.sync.dma_start(wv_t[:, k : k + 2], wv_r[:, k : k + 2])
        if k == 0:
            # split chunk 0 so the first V matmuls (lhsT = t-chunks 0-4)
            # don't wait for the full chunk; the first piece issues from the
            # ACT hwdge queue, whose prologue races the sync queue's
            nc.scalar.dma_start(xT_t[:, 0, 0:512], xT_r[:, 0, 0:512])
            nc.sync.dma_start(xT_t[:, 0, 512:T], xT_r[:, 0, 512:T])
        else:
            nc.sync.dma_start(xT_t[:, k], xT_r[:, k])
        if k == 2:
            # head-0 projection weights early (their psQK chains are 3 of the
            # 8 accumulation streams that pace the rest of the xT stream) --
            # split in halves so each insertion into the chunk stream is small
            nc.sync.dma_start(wq_ts[0][:, 0:8], wq_r[0][:, 0:8])
            nc.sync.dma_start(wk_ts[0][:, 0:8], wk_r[0][:, 0:8])
        if k == 5:
            nc.sync.dma_start(wq_ts[0][:, 8:16], wq_r[0][:, 8:16])
            nc.sync.dma_start(wk_ts[0][:, 8:16], wk_r[0][:, 8:16])
    # heads 1-3 weights after the bulk stream (DMA is idle from here on)
    for h in range(1, H_PER):
        nc.sync.dma_start(wq_ts[h][:], wq_r[h])
        nc.sync.dma_start(wk_ts[h][:], wk_r[h])
    # k-quarter accumulation: 4-chunk chains finish and rotate their PSUM
    # bank, so once chunk 3 lands there is a deep backlog of ungated V work
    # (plus a DVE add per t-chunk per extra quarter) to hide the rest of the
    # xT stream -- full-16-chunk chains would cap concurrent work at 8 chains
    # for the whole stream and starve the PE behind the DMA cadence
    KH = KC // 4
    for half in range(4):
        for tch in range(NTCH):
            ps = psV.tile(
                [P, GCOLS], F32, tag=f"pv{tch % 5}", name=f"{R}psv_{half}_{tch}"
            )
            for kk in range(KH):
                k = half * KH + kk
                nc.tensor.matmul(
                    ps[:],
                    xT_t[:, k, tch * P : (tch + 1) * P],
                    wv_t[:, k],
                    start=(kk == 0),
                    stop=(kk == KH - 1),
                )
            if half == 0:
                nc.vector.tensor_copy(out=v_all[:, tch], in_=ps[:])
            else:
                nc.vector.tensor_add(
                    out=v_all[:, tch], in0=v_all[:, tch], in1=ps[:]
                )
    psV.release()

    # ---------- phase 2: per-head Q^T/K^T projection + attention ----------
    with (
        tc.tile_pool(name=f"{R}psS", bufs=3, space="PSUM") as psS,
        tc.tile_pool(name=f"{R}psC", bufs=1, space="PSUM") as psC,
        tc.tile_pool(name=f"{R}psZ", bufs=1, space="PSUM") as psZ,
    ):
        for h in range(H_PER):
            hs = slice(h * DH, (h + 1) * DH)
            qT_t = qk_pool.tile([P, T], F16, tag="qT", name=f"{R}qT_{h}")
            kT_t = qk_pool.tile([P, T], F16, tag="kT", name=f"{R}kT_{h}")
            wq_t = wq_ts[h]
            wk_t = wk_ts[h]
            if h == 2:
                # wo load once DMA is quiet (mid phase 2), well before phase 3
                nc.sync.dma_start(wo_t[:], wo_r[:])
            # (q,ti),(k,ti) interleaved so attention tile ti only waits for
            # its own two projection chains, not all q chains first
            for ti in range(NTT):
                tsl = slice(ti * TT, (ti + 1) * TT)
                for w_t, dst, nm in ((wq_t, qT_t, "q"), (wk_t, kT_t, "k")):
                    ps = psQK.tile([P, TT], F32, tag="qk", name=f"{R}ps{nm}_{h}_{ti}")
                    for k in range(KC):
                        nc.tensor.matmul(
                            ps[:],
                            w_t[:, k],
                            xT_t[:, k, tsl],
                            start=(k == 0),
                            stop=(k == KC - 1),
                        )
                    nc.vector.tensor_copy(out=dst[:, tsl], in_=ps[:])

            # attention for head h
            for ti in range(NTT):
                tsl = slice(ti * TT, (ti + 1) * TT)
                nch = H_PER * (ti + 1)   # active tk chunks (causal)
                cT_ps = psC.tile([P, TT], F32, tag="c", name=f"{R}c_{h}_{ti}")
                z_ps = psZ.tile([P, TT], F32, tag="z", name=f"{R}z_{h}_{ti}")

                def post(ci, s_ps, h=h, ti=ti, nch=nch, cT_ps=cT_ps, z_ps=z_ps,
                         hs=hs):
                    # diagonal chunks: columns < rel*P are fully masked ->
                    # compute only the live suffix [off:TT] (ragged widths)
                    rel = ci - (nch - H_PER)
                    off = rel * P if rel > 0 else 0
                    rg = slice(off, TT)
                    p_sb = work_pool.tile(
                        [P, TT], F16, tag="w", name=f"{R}p_{h}_{ti}_{ci}"
                    )
                    nc.scalar.activation(
                        p_sb[:, rg],
                        s_ps[:, rg],
                        mybir.ActivationFunctionType.Exp,
                        scale=SCALE,
                    )
                    if rel >= 0:
                        # triangular 128-wide band at the suffix start; the
                        # band-local mask is mask[p, jj] = (jj >= p)
                        nc.vector.tensor_mul(
                            out=p_sb[:, off : off + P],
                            in0=p_sb[:, off : off + P],
                            in1=band_mask,
                        )
                    nc.tensor.matmul(
                        cT_ps[:, rg],
                        v_all[:, ci, hs],
                        p_sb[:, rg],
                        start=(ci == 0),
                        stop=(ci == nch - 1),
                        skip_group_check=True,
                    )
                    nc.tensor.matmul(
                        z_ps[:, rg],
                        ones_t[:],
                        p_sb[:, rg],
                        start=(ci == 0),
                        stop=(ci == nch - 1),
                        skip_group_check=True,
                    )

                pending = None
                for ci in range(nch):
                    srel = ci - (nch - H_PER)
                    soff = srel * P if srel > 0 else 0
                    s_ps = psS.tile([P, TT], F32, tag="s", name=f"{R}s_{h}_{ti}_{ci}")
                    nc.tensor.matmul(
                        s_ps[:, soff:TT],
                        kT_t[:, ci * P : (ci + 1) * P],
                        qT_t[:, ti * TT + soff : (ti + 1) * TT],
                        start=True,
                        stop=True,
                    )
                    if pending is not None:
                        post(*pending)
                    pending = (ci, s_ps)
                post(*pending)

                if h < H_PER - 1:
                    recip = work_pool.tile(
                        [P, TT], F32, tag="rc", name=f"{R}rc_{h}_{ti}"
                    )
                    nc.vector.reciprocal(recip[:], z_ps[:])
                    nc.vector.tensor_mul(
                        out=ct_sb[:, h, tsl], in0=cT_ps[:], in1=recip[:]
                    )
                else:
                    # head 3: normalize per 128-wide tq chunk, and emit that
                    # chunk's output-projection burst immediately after its
                    # norm (phase-3 interleaved per tq chunk) -- this keeps
                    # each burst's PSUM-evacuation copies ahead of the NEXT
                    # chunk's norm in the DVE queue, so the 3-bank psQK
                    # rotation never waits on queued normalization work
                    for tqi in range(NTT):
                        sub = tqi
                        psl = slice(sub * P, (sub + 1) * P)
                        recip = work_pool.tile(
                            [P, P], F32, tag="rc", name=f"{R}rc_{h}_{ti}_{sub}"
                        )
                        nc.vector.reciprocal(recip[:], z_ps[:, psl])
                        nc.vector.tensor_mul(
                            out=ct_sb[:, h, ti * TT + sub * P : ti * TT + (sub + 1) * P],
                            in0=cT_ps[:, psl],
                            in1=recip[:],
                        )
                        tq = ti * NTT + tqi
                        for do in range(NDOT):
                            dsl = slice(do * DOT, (do + 1) * DOT)
                            ps = psQK.tile(
                                [P, TT], F32, tag="qk", name=f"{R}po_{tq}_{do}"
                            )
                            for hh in range(H_PER):
                                nc.tensor.matmul(
                                    ps[:],
                                    ct_sb[:, hh, tq * P : (tq + 1) * P],
                                    wo_t[:, hh, dsl],
                                    start=(hh == 0),
                                    stop=(hh == H_PER - 1),
                                )
                            ost = work_pool.tile(
                                [P, DOT], F16, tag="w", name=f"{R}ost_{tq}_{do}"
                            )
                            # PSUM->SBUF evacuation alternates DVE / ACT so
                            # neither engine gates the 853ns/tile PE cadence;
                            # the final tile goes to DVE, which is idle by
                            # then (ACT still drains its exp/copy queue)
                            if (do + tqi + (ti == NTT - 1)) % 2 == 0 or (
                                tq == NTCH - 1 and do == NDOT - 1
                            ):
                                nc.vector.tensor_copy(out=ost[:], in_=ps[:])
                            else:
                                nc.scalar.activation(
                                    ost[:],
                                    ps[:],
                                    mybir.ActivationFunctionType.Copy,
                                )
                            nc.sync.dma_start(
                                out_r[tq * P : (tq + 1) * P, dsl], ost[:]
                            )

    psQK.release()


def _get_nc(n_repeat=1):
    key = f"nc{n_repeat}"
    if key not in _CACHE:
        _CACHE[key] = _build(n_repeat)
    return _CACHE[key]


def _pack_heads(w):
    """(D, GCOLS) fp16 -> (H_PER, P, KC, DH): per-head SBUF tile layout
    [d%128 partition, d//128 chunk, dh] so the DMA is 4KB-contiguous."""
    # (D, H_PER, DH) -> (KC, P, H_PER, DH) -> (H_PER, P, KC, DH)
    r = w.reshape(KC, P, H_PER, DH).transpose(2, 1, 0, 3)
    return np.ascontiguousarray(r)


def make_in_maps(x, q_out, k_out, v_out, w_out):
    q16 = q_out.astype(np.float16)
    k16 = k_out.astype(np.float16)
    v16 = v_out.astype(np.float16)
    w16 = w_out.astype(np.float16)
    in_maps = []
    for b in range(B):
        xT = np.ascontiguousarray(x[b].T.astype(np.float16))
        for g in range(4):  # head groups
            cols = slice(g * GCOLS, (g + 1) * GCOLS)
            in_maps.append(
                {
                    "xT": xT,
                    "wq": _pack_heads(q16[:, cols]),
                    "wk": _pack_heads(k16[:, cols]),
                    "wv": np.ascontiguousarray(v16[:, cols]),
                    "wo": np.ascontiguousarray(w16[cols, :]),
                }
            )
    return in_maps


def kernel(**inputs) -> np.ndarray:
    x = np.ascontiguousarray(np.asarray(inputs["x"], dtype=np.float32))
    q_out = np.ascontiguousarray(np.asarray(inputs["q_out"], dtype=np.float32))
    k_out = np.ascontiguousarray(np.asarray(inputs["k_out"], dtype=np.float32))
    v_out = np.ascontiguousarray(np.asarray(inputs["v_out"], dtype=np.float32))
    w_out = np.ascontiguousarray(np.asarray(inputs["w_out"], dtype=np.float32))

    nc = _get_nc()
    in_maps = make_in_maps(x, q_out, k_out, v_out, w_out)
    res = run_bass_kernel_spmd(nc, in_maps, core_ids=list(range(8)))
    outs = [res.results[c]["out"].astype(np.float32) for c in range(8)]
    full = np.stack(
        [
            outs[0] + outs[1] + outs[2] + outs[3],
            outs[4] + outs[5] + outs[6] + outs[7],
        ]
    )
    return full.astype(np.float32)



# revision 5
# speedup vs baseline: 1.0648x; 1.0648x over previous
"""Distributed causal multi-head attention for Trainium2 (8 NeuronCores).

Problem (nn_Attention): B=2, T=2048, D=2048, H=16 heads, d_head=128.
  q/k/v = x @ {q,k,v}_out; per-head causal softmax attention; out = ctx @ w_out.

Sharding: batch (2) x head-group (4 heads each) -> 8 cores. Each core computes
its batch's attention for its 4 heads plus the partial output projection
(w_out row-sharded); the host sums the 4 partials per batch (all-reduce) and
stacks batches.

All device tensors are fp16 (host converts): PE rate is identical to fp32r
(1 cyc/row) but every DMA halves, the <256-wide fp32r penalty disappears,
and the whole working set (x^T, V, Q^T/K^T, all four heads' context, w_out
slice) fits in SBUF -- no DRAM scratch round-trip. PSUM accumulation stays
fp32 throughout, so precision loss is only input/intermediate quantization
(~1e-3 rel).

Per-core kernel (all matmuls fp16, moving free dim 512):
  phase 1: V  = x @ wv   (single pass, wv resident, all 4 heads at once)
  phase 2: per head: Q^T, K^T projections; then flash-style causal attention
           with scores kept transposed (tk on partitions):
             S^T chunk = K^T_chunk.T @ Q^T_tile        (PE)
             P^T = exp(S^T / sqrt(dh))                 (ACT, PSUM->SBUF, f16)
             diagonal chunks masked via precomputed 0/1 mask    (DVE)
             C^T += V_chunk.T @ P^T ; Z += ones.T @ P^T (PE, PSUM accum)
             C^T_norm = C^T * 1/Z -> SBUF ct_sb        (DVE)
  phase 3: out[tq, :] += sum_h C_h @ wo_h  (PSUM accum over heads),
           interleaved with head-3 attention per tq-group.

Host passes x^T (per batch) so the contraction dim D is on partitions
everywhere; no on-device transposes needed anywhere.
"""

import math

import numpy as np

import concourse.bacc as bacc
import concourse.mybir as mybir
import concourse.tile as tile
from concourse.bass_utils import run_bass_kernel_spmd

# ---- problem constants (hardcoded; self-contained) ----
B = 2
T = 2048
D = 2048
H_PER = 4            # heads per core
DH = 128             # head dim
GCOLS = H_PER * DH   # 512 columns per head-group
P = 128
KC = D // P          # 16 contraction chunks
TT = 512             # t tile (matmul moving free dim)
NTT = T // TT        # 4
NTCH = T // P        # 16 t chunks
DOT = 512            # output-dim tile
NDOT = D // DOT      # 4

F32 = mybir.dt.float32
F16 = mybir.dt.float16
SCALE = 1.0 / math.sqrt(float(DH))

_CACHE = {}


def _build(n_repeat=1):
    nc = bacc.Bacc("TRN2", target_bir_lowering=False, debug=False)
    xT_d = nc.dram_tensor("xT", (D, T), F16, kind="ExternalInput")
    # wq/wk are host-packed per head into the exact SBUF tile layout
    # (p, c, dh): a column-sliced (D, 128) load would have 256B descriptors
    # and pay the <512B DMA read-modify-write penalty
    wq_d = nc.dram_tensor("wq", (H_PER, P, KC, DH), F16, kind="ExternalInput")
    wk_d = nc.dram_tensor("wk", (H_PER, P, KC, DH), F16, kind="ExternalInput")
    wv_d = nc.dram_tensor("wv", (D, GCOLS), F16, kind="ExternalInput")
    wo_d = nc.dram_tensor("wo", (GCOLS, D), F16, kind="ExternalInput")
    out_d = nc.dram_tensor("out", (T, D), F16, kind="ExternalOutput")

    xT_r = xT_d.ap().rearrange("(c p) t -> p c t", p=P)      # (128, 16, 2048)
    wq_r = wq_d.ap()                                         # (4, 128, 16, 128)
    wk_r = wk_d.ap()
    wv_r = wv_d.ap().rearrange("(c p) g -> p c g", p=P)
    wo_r = wo_d.ap().rearrange("(h p) n -> p h n", p=P)      # (128, 4, 2048)
    out_r = out_d.ap()

    with tile.TileContext(nc) as tc:
        with (
            tc.tile_pool(name="const", bufs=1) as const_pool,
            tc.tile_pool(name="big", bufs=1) as big_pool,
            tc.tile_pool(name="wvp", bufs=1) as wv_pool,
            tc.tile_pool(name="vp", bufs=1) as v_pool,
            tc.tile_pool(name="ctp", bufs=1) as ct_pool,
            tc.tile_pool(name="wop", bufs=1) as wo_pool,
            tc.tile_pool(name="qk", bufs=2) as qk_pool,
            tc.tile_pool(name="wqk", bufs=4) as wqk_pool,
            tc.tile_pool(name="work", bufs=6) as work_pool,
            tc.tile_pool(name="accp", bufs=2) as acc_pool,
        ):
            # ---- constants ----
            # ones for the Z (softmax denominator) matmuls: generated on the
            # idle Pool engine instead of DMA'd, keeping the critical phase-1
            # chunk stream free of an extra HWDGE dispatch
            ones_t = const_pool.tile([P, P], F16, tag="ones", name="ones_t")
            nc.gpsimd.memset(ones_t[:], 1.0)
            # band-local triangular mask: band[p, jj] = 1.0 iff jj >= p
            # (ragged diagonal chunks only ever mask a 128-wide band)
            bandt = const_pool.tile([P, P], F32, tag="mask", name="bandt")
            nc.gpsimd.memset(bandt[:], 1.0)
            nc.gpsimd.affine_select(
                out=bandt[:],
                in_=bandt[:],
                compare_op=mybir.AluOpType.is_ge,
                fill=0.0,
                base=0,
                pattern=[[1, P]],
                channel_multiplier=-1,
            )
            band16 = const_pool.tile([P, P], F16, tag="mask16", name="band16")
            nc.vector.tensor_copy(out=band16[:], in_=bandt[:])
            band_mask = band16[:]

            # warm the ACT exp table during phase 1 (LoadActFuncSet is ~1.3us
            # and otherwise stalls the first real exp)
            actwarm = const_pool.tile([P, 1], F32, tag="actwarm", name="actwarm")
            nc.scalar.activation(
                actwarm[:], bandt[:, 0:1], mybir.ActivationFunctionType.Exp
            )

            for rep in range(n_repeat):
                _emit_body(
                    nc, tc, rep, big_pool, wv_pool, v_pool, ct_pool, wo_pool,
                    qk_pool, wqk_pool, work_pool, acc_pool,
                    ones_t, band_mask,
                    xT_r, wq_r, wk_r, wv_r, wo_r, out_r,
                )

    nc.compile()
    return nc


def _emit_body(nc, tc, rep, big_pool, wv_pool, v_pool, ct_pool, wo_pool,
               qk_pool, wqk_pool, work_pool, acc_pool,
               ones_t, band_mask,
               xT_r, wq_r, wk_r, wv_r, wo_r, out_r):
    R = f"r{rep}_"

    # ---- resident tensors ----
    xT_t = big_pool.tile([P, KC, T], F16, tag="big", name=f"{R}xT_t")
    wv_t = wv_pool.tile([P, KC, GCOLS], F16, tag="wv", name=f"{R}wv_t")
    v_all = v_pool.tile([P, NTCH, GCOLS], F16, tag="v", name=f"{R}v_all")
    ct_sb = ct_pool.tile([P, H_PER, T], F16, tag="ct", name=f"{R}ct_sb")
    wo_t = wo_pool.tile([P, H_PER, D], F16, tag="wo", name=f"{R}wo_t")

    # psQK doubles as the phase-3 accumulator pool (same tag/shape rotates)
    psQK = tc.alloc_tile_pool(name=f"{R}psQK", bufs=3, space="PSUM")

    # per-head weight tiles; all loads on the sync queue at controlled
    # positions so they never cut ahead of the critical phase-1 chunk stream
    wq_ts, wk_ts = [], []
    for h in range(H_PER):
        wq_ts.append(wqk_pool.tile([P, KC, DH], F16, tag="wq", name=f"{R}wq_{h}"))
        wk_ts.append(wqk_pool.tile([P, KC, DH], F16, tag="wk", name=f"{R}wk_{h}"))

    # ---------- phase 1: V = x @ wv (single pass, wv resident) ----------
    psV = tc.alloc_tile_pool(name=f"{R}psV", bufs=1, space="PSUM")
    for k in range(KC):
        # interleave wv + xT chunk loads so phase-1 matmuls can start
        # as soon as the first chunks land (wv batched in pairs after the
        # first chunks: fewer HWDGE dispatch slots in the critical stream)
        if k < 2:
            nc.sync.dma_start(wv_t[:, k], wv_r[:, k])
        elif k % 2 == 0:
            nc.sync.dma_start(wv_t[:, k : k + 2], wv_r[:, k : k + 2])
        if k == 0:
            # split chunk 0 so the first V matmuls (lhsT = t-chunks 0-4)
            # don't wait for the full chunk; the first piece issues from the
            # ACT hwdge queue, whose prologue races the sync queue's
            nc.scalar.dma_start(xT_t[:, 0, 0:512], xT_r[:, 0, 0:512])
            nc.sync.dma_start(xT_t[:, 0, 512:T], xT_r[:, 0, 512:T])
        else:
            nc.sync.dma_start(xT_t[:, k], xT_r[:, k])
        if k == 2:
            # head-0 projection weights early (their psQK chains are 3 of the
            # 8 accumulation streams that pace the rest of the xT stream) --
            # split in halves so each insertion into the chunk stream is small
            nc.sync.dma_start(wq_ts[0][:, 0:8], wq_r[0][:, 0:8])
            nc.sync.dma_start(wk_ts[0][:, 0:8], wk_r[0][:, 0:8])
        if k == 5:
            nc.sync.dma_start(wq_ts[0][:, 8:16], wq_r[0][:, 8:16])
            nc.sync.dma_start(wk_ts[0][:, 8:16], wk_r[0][:, 8:16])
    # heads 1-3 weights after the bulk stream (DMA is idle from here on)
    for h in range(1, H_PER):
        nc.sync.dma_start(wq_ts[h][:], wq_r[h])
        nc.sync.dma_start(wk_ts[h][:], wk_r[h])
    # k-quarter accumulation: 4-chunk chains finish and rotate their PSUM
    # bank, so once chunk 3 lands there is a deep backlog of ungated V work
    # (plus a DVE add per t-chunk per extra quarter) to hide the rest of the
    # xT stream -- full-16-chunk chains would cap concurrent work at 8 chains
    # for the whole stream and starve the PE behind the DMA cadence
    KH = KC // 4
    for half in range(4):
        for tch in range(NTCH):
            ps = psV.tile(
                [P, GCOLS], F32, tag=f"pv{tch % 5}", name=f"{R}psv_{half}_{tch}"
            )
            for kk in range(KH):
                k = half * KH + kk
                nc.tensor.matmul(
                    ps[:],
                    xT_t[:, k, tch * P : (tch + 1) * P],
                    wv_t[:, k],
                    start=(kk == 0),
                    stop=(kk == KH - 1),
                )
            if half == 0:
                nc.vector.tensor_copy(out=v_all[:, tch], in_=ps[:])
            else:
                nc.vector.tensor_add(
                    out=v_all[:, tch], in0=v_all[:, tch], in1=ps[:]
                )
    psV.release()

    # ---------- phase 2: per-head Q^T/K^T projection + attention ----------
    with (
        tc.tile_pool(name=f"{R}psS", bufs=3, space="PSUM") as psS,
        tc.tile_pool(name=f"{R}psC", bufs=1, space="PSUM") as psC,
        tc.tile_pool(name=f"{R}psZ", bufs=1, space="PSUM") as psZ,
    ):
        for h in range(H_PER):
            hs = slice(h * DH, (h + 1) * DH)
            qT_t = qk_pool.tile([P, T], F16, tag="qT", name=f"{R}qT_{h}")
            kT_t = qk_pool.tile([P, T], F16, tag="kT", name=f"{R}kT_{h}")
            wq_t = wq_ts[h]
            wk_t = wk_ts[h]
            if h == 2:
                # wo load once DMA is quiet (mid phase 2), well before phase 3
                nc.sync.dma_start(wo_t[:], wo_r[:])
            # (q,ti),(k,ti) interleaved so attention tile ti only waits for
            # its own two projection chains, not all q chains first
            for ti in range(NTT):
                tsl = slice(ti * TT, (ti + 1) * TT)
                for w_t, dst, nm in ((wq_t, qT_t, "q"), (wk_t, kT_t, "k")):
                    ps = psQK.tile([P, TT], F32, tag="qk", name=f"{R}ps{nm}_{h}_{ti}")
                    for k in range(KC):
                        nc.tensor.matmul(
                            ps[:],
                            w_t[:, k],
                            xT_t[:, k, tsl],
                            start=(k == 0),
                            stop=(k == KC - 1),
                        )
                    nc.vector.tensor_copy(out=dst[:, tsl], in_=ps[:])

            # attention for head h
            for ti in range(NTT):
                tsl = slice(ti * TT, (ti + 1) * TT)
                nch = H_PER * (ti + 1)   # active tk chunks (causal)
                cT_ps = psC.tile([P, TT], F32, tag="c", name=f"{R}c_{h}_{ti}")
                z_ps = psZ.tile([P, TT], F32, tag="z", name=f"{R}z_{h}_{ti}")
                # fp16 running sum of P chunks on DVE (partition-wise); the
                # softmax denominator then needs only ONE 512-moving ones
                # matmul per (h, ti) instead of one per chunk -- saves ~61k
                # PE cycles (~25us) across the kernel. fp16 accumulation adds
                # ~1e-3 rel err to Z (16 sequential adds), well within gate.
                acc_sb = acc_pool.tile([P, TT], F16, tag="acc", name=f"{R}acc_{h}_{ti}")

                def post(ci, s_ps, h=h, ti=ti, nch=nch, cT_ps=cT_ps,
                         acc_sb=acc_sb, hs=hs):
                    # diagonal chunks: columns < rel*P are fully masked ->
                    # compute only the live suffix [off:TT] (ragged widths)
                    rel = ci - (nch - H_PER)
                    off = rel * P if rel > 0 else 0
                    rg = slice(off, TT)
                    p_sb = work_pool.tile(
                        [P, TT], F16, tag="w", name=f"{R}p_{h}_{ti}_{ci}"
                    )
                    nc.scalar.activation(
                        p_sb[:, rg],
                        s_ps[:, rg],
                        mybir.ActivationFunctionType.Exp,
                        scale=SCALE,
                    )
                    if rel >= 0:
                        # triangular 128-wide band at the suffix start; the
                        # band-local mask is mask[p, jj] = (jj >= p)
                        nc.vector.tensor_mul(
                            out=p_sb[:, off : off + P],
                            in0=p_sb[:, off : off + P],
                            in1=band_mask,
                        )
                    nc.tensor.matmul(
                        cT_ps[:, rg],
                        v_all[:, ci, hs],
                        p_sb[:, rg],
                        start=(ci == 0),
                        stop=(ci == nch - 1),
                        skip_group_check=True,
                    )
                    if ci == 0:
                        nc.vector.tensor_copy(out=acc_sb[:], in_=p_sb[:])
                    else:
                        nc.vector.tensor_add(
                            out=acc_sb[:, rg], in0=acc_sb[:, rg], in1=p_sb[:, rg]
                        )

                pending = None
                for ci in range(nch):
                    srel = ci - (nch - H_PER)
                    soff = srel * P if srel > 0 else 0
                    s_ps = psS.tile([P, TT], F32, tag="s", name=f"{R}s_{h}_{ti}_{ci}")
                    nc.tensor.matmul(
                        s_ps[:, soff:TT],
                        kT_t[:, ci * P : (ci + 1) * P],
                        qT_t[:, ti * TT + soff : (ti + 1) * TT],
                        start=True,
                        stop=True,
                    )
                    if pending is not None:
                        post(*pending)
                    pending = (ci, s_ps)
                post(*pending)
                # single partition-sum matmul over the accumulated P
                nc.tensor.matmul(
                    z_ps[:], ones_t[:], acc_sb[:], start=True, stop=True
                )

                if h < H_PER - 1:
                    recip = work_pool.tile(
                        [P, TT], F32, tag="rc", name=f"{R}rc_{h}_{ti}"
                    )
                    nc.vector.reciprocal(recip[:], z_ps[:])
                    nc.vector.tensor_mul(
                        out=ct_sb[:, h, tsl], in0=cT_ps[:], in1=recip[:]
                    )
                else:
                    # head 3: normalize per 128-wide tq chunk, and emit that
                    # chunk's output-projection burst immediately after its
                    # norm (phase-3 interleaved per tq chunk) -- this keeps
                    # each burst's PSUM-evacuation copies ahead of the NEXT
                    # chunk's norm in the DVE queue, so the 3-bank psQK
                    # rotation never waits on queued normalization work
                    for tqi in range(NTT):
                        sub = tqi
                        psl = slice(sub * P, (sub + 1) * P)
                        recip = work_pool.tile(
                            [P, P], F32, tag="rc", name=f"{R}rc_{h}_{ti}_{sub}"
                        )
                        nc.vector.reciprocal(recip[:], z_ps[:, psl])
                        nc.vector.tensor_mul(
                            out=ct_sb[:, h, ti * TT + sub * P : ti * TT + (sub + 1) * P],
                            in0=cT_ps[:, psl],
                            in1=recip[:],
                        )
                        tq = ti * NTT + tqi
                        for do in range(NDOT):
                            dsl = slice(do * DOT, (do + 1) * DOT)
                            ps = psQK.tile(
                                [P, TT], F32, tag="qk", name=f"{R}po_{tq}_{do}"
                            )
                            for hh in range(H_PER):
                                nc.tensor.matmul(
                                    ps[:],
                                    ct_sb[:, hh, tq * P : (tq + 1) * P],
                                    wo_t[:, hh, dsl],
                                    start=(hh == 0),
                                    stop=(hh == H_PER - 1),
                                )
                            ost = work_pool.tile(
                                [P, DOT], F16, tag="w", name=f"{R}ost_{tq}_{do}"
                            )
                            # PSUM->SBUF evacuation alternates DVE / ACT so
                            # neither engine gates the 853ns/tile PE cadence;
                            # the final tile goes to DVE, which is idle by
                            # then (ACT still drains its exp/copy queue)
                            if (do + tqi + (ti == NTT - 1)) % 2 == 0 or (
                                tq == NTCH - 1 and do == NDOT - 1
                            ):
                                nc.vector.tensor_copy(out=ost[:], in_=ps[:])
                            else:
                                nc.scalar.activation(
                                    ost[:],
                                    ps[:],
                                    mybir.ActivationFunctionType.Copy,
                                )
                            nc.sync.dma_start(
                                out_r[tq * P : (tq + 1) * P, dsl], ost[:]
                            )

    psQK.release()


def _get_nc(n_repeat=1):
    key = f"nc{n_repeat}"
    if key not in _CACHE:
        _CACHE[key] = _build(n_repeat)
    return _CACHE[key]


def _pack_heads(w):
    """(D, GCOLS) fp16 -> (H_PER, P, KC, DH): per-head SBUF tile layout
    [d%128 partition, d//128 chunk, dh] so the DMA is 4KB-contiguous."""
    # (D, H_PER, DH) -> (KC, P, H_PER, DH) -> (H_PER, P, KC, DH)
    r = w.reshape(KC, P, H_PER, DH).transpose(2, 1, 0, 3)
    return np.ascontiguousarray(r)


def make_in_maps(x, q_out, k_out, v_out, w_out):
    q16 = q_out.astype(np.float16)
    k16 = k_out.astype(np.float16)
    v16 = v_out.astype(np.float16)
    w16 = w_out.astype(np.float16)
    in_maps = []
    for b in range(B):
        xT = np.ascontiguousarray(x[b].T.astype(np.float16))
        for g in range(4):  # head groups
            cols = slice(g * GCOLS, (g + 1) * GCOLS)
            in_maps.append(
                {
                    "xT": xT,
                    "wq": _pack_heads(q16[:, cols]),
                    "wk": _pack_heads(k16[:, cols]),
                    "wv": np.ascontiguousarray(v16[:, cols]),
                    "wo": np.ascontiguousarray(w16[cols, :]),
                }
            )
    return in_maps


def kernel(**inputs) -> np.ndarray:
    x = np.ascontiguousarray(np.asarray(inputs["x"], dtype=np.float32))
    q_out = np.ascontiguousarray(np.asarray(inputs["q_out"], dtype=np.float32))
    k_out = np.ascontiguousarray(np.asarray(inputs["k_out"], dtype=np.float32))
    v_out = np.ascontiguousarray(np.asarray(inputs["v_out"], dtype=np.float32))
    w_out = np.ascontiguousarray(np.asarray(inputs["w_out"], dtype=np.float32))

    nc = _get_nc()
    in_maps = make_in_maps(x, q_out, k_out, v_out, w_out)
    res = run_bass_kernel_spmd(nc, in_maps, core_ids=list(range(8)))
    outs = [res.results[c]["out"].astype(np.float32) for c in range(8)]
    full = np.stack(
        [
            outs[0] + outs[1] + outs[2] + outs[3],
            outs[4] + outs[5] + outs[6] + outs[7],
        ]
    )
    return full.astype(np.float32)

